# revision 4
# baseline (speedup 1.0000x reference)
"""Trainium2 Bass kernel for nn_ChannelAdaptiveNormalization (v2).

Reference computation (per batch):
    src_n = instnorm(src); q = Wq@src_n; k = Wk@instnorm(trg); v = Wv@trg
    attn = softmax(q^T k / sqrt(C))  over t
    mean = attn @ v ; var = relu(attn @ v^2 - mean^2)
    out = sqrt(mean_s[var]) * src_n + mean_s[mean]      (broadcast over time)

v2 design (per-core, data-parallel over batch, 2 batches/core):
  * instance-norm folded into CxC projection weights (column scale by 1/sd,
    rank-1 bias) -- normalized activations never materialize.
  * q/k/p/v/v^2 stored fp8e4; all attention matmuls (scores^T, Z, attn@v,
    attn@v^2) run in DoubleRow perf mode (contract 256 per instruction at
    0.5 cyc/row) -- 2x PE throughput.
  * attention is kept UNNORMALIZED through the PE stages: p = exp(s/16-3)
    (the shift keeps fp8e4 under its +-240 max and cancels in normalization);
    Z via a DoubleRow ones-matmul; 1/Z folded in only at the tiny [d,s]
    AV/AV2 outputs (8x fewer elements than normalizing p).
  * only column-sums over s of M=attn@v, M^2, attn@v^2 are needed; they are
    reduced on the fly from PSUM with accumulate ops.
  * all sqrt computed as exp(0.5*ln(x)) so the ACT engine keeps ONE table
    (natural_log_exp) resident: zero table swaps.
  * final output is a per-(b,c) affine of raw fp32 src: out = A*src + B.
"""

import sys

import numpy as np

if "/opt/trn_rl_repo" not in sys.path:
    sys.path.insert(0, "/opt/trn_rl_repo")

from contextlib import ExitStack

import concourse.bass as bass
import concourse.tile as tile
from concourse import mybir
from concourse.bass_utils import run_bass_kernel_spmd

DT = mybir.dt
ALU = mybir.AluOpType
ACTF = mybir.ActivationFunctionType
DR = mybir.MatmulPerfMode.DoubleRow

N_CORES = 8
B_FULL = 16
B_SH = B_FULL // N_CORES  # 2 batches per core
C = 256
T = 2048
P = 128
NCH = C // P  # 2 channel chunks
NTCH = T // P  # 16 time chunks
NQ = 4  # s-quarters
SQ = T // NQ  # 512 s per quarter
NPAIR = NTCH // 2  # 8 tchunk pairs (DoubleRow contracts 2 chunks)


def _build_nc() -> bass.Bass:
    nc = bass.Bass()
    src = nc.declare_dram_parameter("src", [B_SH, C, T], DT.float32, isOutput=False)
    trg = nc.declare_dram_parameter("trg", [B_SH, C, T], DT.float32, isOutput=False)
    wqt = nc.declare_dram_parameter("wqt", [C, C], DT.float32, isOutput=False)
    wkt = nc.declare_dram_parameter("wkt", [C, C], DT.float32, isOutput=False)
    wvt = nc.declare_dram_parameter("wvt", [C, C], DT.float32, isOutput=False)
    out = nc.declare_dram_parameter("out", [B_SH, C, T], DT.float32, isOutput=True)

    with tile.TileContext(nc) as tc:
        with ExitStack() as ctx:
            _build_kernel(ctx, tc, src, trg, wqt, wkt, wvt, out)
    _legalize_waits(nc)
    return nc


def _legalize_waits(nc: bass.Bass):
    """walrus on this toolchain encodes at most ONE sync wait per
    instruction (NEURON_ISA_TPB_EVENTS has a single wait slot and no
    splitting pass runs).  Hoist all but the last wait of every
    instruction into standalone single-wait EventSemaphore instructions
    on the same engine queue, which preserves ordering semantics."""
    all_sems = {}
    for fn in nc.m.functions:
        for blk in fn.blocks:
            for inst in blk.instructions:
                si = getattr(inst, "sync_info", None)
                if si is None:
                    continue
                for w in list(si.on_wait) + list(si.on_update):
                    if not w.ant_name.startswith("barrier"):
                        all_sems[w.id] = w.ant_name

    for fn in nc.m.functions:
        for blk in fn.blocks:
            snapshot = list(blk.instructions)
            for idx in range(len(snapshot) - 1, -1, -1):
                inst = snapshot[idx]
                if type(inst).__name__ == "InstISA" and getattr(inst, "isa_opcode", None) == 176:
                    # EVENT_SEMAPHORE_RANGE_CLEAR: encoding mismatches this
                    # walrus build; replace with per-sem zero-writes.
                    pos = list(blk.instructions).index(inst)
                    blk.instructions.pop(pos)
                    for sid, sname in sorted(all_sems.items()):
                        ev = mybir.InstEventSemaphore(
                            name=nc.get_next_instruction_name(), ins=[], outs=[]
                        )
                        ev.engine = inst.engine
                        ev.sync_info = mybir.SyncInfo(
                            on_wait=[],
                            on_update=[
                                mybir.SyncUpdate(
                                    sync_type="semaphore",
                                    id=sid,
                                    ant_name=sname,
                                    update_mode="sem-wr-imm",
                                    update_value=0,
                                )
                            ],
                        )
                        nc.register_instruction(ev)
                        blk.instructions.insert(pos, ev)
                        pos += 1

    for fn in nc.m.functions:
        for blk in fn.blocks:
            snapshot = list(blk.instructions)
            for idx in range(len(snapshot) - 1, -1, -1):
                inst = snapshot[idx]
                si = getattr(inst, "sync_info", None)
                if si is None or len(si.on_wait) <= 1:
                    continue
                waits = list(si.on_wait)
                evs = []
                for w in waits[:-1]:
                    ev = mybir.InstEventSemaphore(
                        name=nc.get_next_instruction_name(), ins=[], outs=[]
                    )
                    ev.engine = inst.engine
                    ev.sync_info = mybir.SyncInfo(on_wait=[w], on_update=[])
                    nc.register_instruction(ev)
                    evs.append(ev)
                si.on_wait = waits[-1:]
                inst.sync_info = si
                for ev in reversed(evs):
                    blk.instructions.insert(idx, ev)


def _build_kernel(ctx, tc, src, trg, wqt, wkt, wvt, out):
    nc = tc.nc
    ep = ctx.enter_context

    # ---------------- pools ----------------
    pool_const = ep(tc.tile_pool(name="const", bufs=1))
    pool_io = ep(tc.tile_pool(name="io", bufs=2))        # fp32 inputs
    pool_bf = ep(tc.tile_pool(name="bfx", bufs=2))       # bf16 activations
    pool_qk = ep(tc.tile_pool(name="qk", bufs=2))        # fp8 q/k
    pool_v = ep(tc.tile_pool(name="vp", bufs=2))         # fp8 v/v2
    pool_p = ep(tc.tile_pool(name="pp", bufs=2))         # fp8 exp scores
    pool_st = ep(tc.tile_pool(name="st", bufs=2))        # small stats
    pool_ws = ep(tc.tile_pool(name="ws", bufs=2))        # scaled weights
    pool_zi = ep(tc.tile_pool(name="zi", bufs=2))        # zinv
    pool_jk = ep(tc.tile_pool(name="jk", bufs=2))        # junk evict targets
    pool_out = ep(tc.tile_pool(name="oo", bufs=2))       # output staging
    pool_wtmp = ep(tc.tile_pool(name="wtmp", bufs=3))

    ps_sc = ep(tc.tile_pool(name="ps_sc", bufs=1, space="PSUM"))   # scores [P,1024]
    ps_z = ep(tc.tile_pool(name="ps_z", bufs=1, space="PSUM"))     # Z [P,512]
    ps_avu = ep(tc.tile_pool(name="ps_avu", bufs=2, space="PSUM"))  # AVu per dc
    ps_av2 = ep(tc.tile_pool(name="ps_av2", bufs=2, space="PSUM"))  # AV2u per dc
    ps_pr = ep(tc.tile_pool(name="ps_pr", bufs=1, space="PSUM"))   # projections

    # ---------------- constants / weights (once) ----------------
    ones_f8 = pool_const.tile([P, 2, P], DT.float8e4, name="ones_f8")
    nc.vector.memset(ones_f8[:], 1.0)
    cb_exp = pool_const.tile([P, 1], DT.float32, name="cb_exp")
    nc.vector.memset(cb_exp[:], -3.0)
    cb_tiny = pool_const.tile([P, 1], DT.float32, name="cb_tiny")
    nc.vector.memset(cb_tiny[:], 1e-30)
    cb_zero = pool_const.tile([P, 1], DT.float32, name="cb_zero")
    nc.vector.memset(cb_zero[:], 0.0)

    # weight layout in SBUF: [128 (c within chunk), cc*256 + d]
    wq_bf = pool_const.tile([P, NCH * C], DT.bfloat16, name="wq_bf")
    wk_bf = pool_const.tile([P, NCH * C], DT.bfloat16, name="wk_bf")
    wv_bf = pool_const.tile([P, NCH * C], DT.bfloat16, name="wv_bf")
    for w_bf, w_d in ((wq_bf, wqt), (wk_bf, wkt), (wv_bf, wvt)):
        wtmp = pool_wtmp.tile([P, NCH * C], DT.float32, name="wtmp")
        nc.gpsimd.dma_start(
            wtmp[:].rearrange("p (a d) -> p a d", a=NCH),
            w_d[:].rearrange("(a p) d -> p a d", p=P),
        )
        nc.vector.tensor_copy(w_bf[:], wtmp[:])

    # ---------------- per-batch state ----------------
    class BState:
        pass

    st = [BState() for _ in range(B_SH)]

    def emit_loads(b):
        s = st[b]
        s.s_f, s.t_f = [], []
        for cc in range(NCH):
            sf = pool_io.tile([P, T], DT.float32, name=f"s_f{cc}")
            tf = pool_io.tile([P, T], DT.float32, name=f"t_f{cc}")
            for h in range(2):
                sl = slice(1024 * h, 1024 * (h + 1))
                nc.gpsimd.dma_start(sf[:, sl], src[b, cc * P : (cc + 1) * P, sl])
                nc.gpsimd.dma_start(tf[:, sl], trg[b, cc * P : (cc + 1) * P, sl])
            s.s_f.append(sf)
            s.t_f.append(tf)

    def emit_stats_casts(b):
        """bn stats on fp32 inputs, rsqrt via ln/exp on ACT, bf16 casts on
        Pool, scaled weights + rank-1 bias prep inputs."""
        s = st[b]
        # bf16 casts (Pool) -- needed for projections only
        s.s_bf, s.t_bf = [], []
        for cc in range(NCH):
            sb = pool_bf.tile([P, T], DT.bfloat16, name=f"s_bf{cc}")
            nc.gpsimd.tensor_copy(sb[:], s.s_f[cc][:])
            s.s_bf.append(sb)
        for cc in range(NCH):
            tb = pool_bf.tile([P, T], DT.bfloat16, name=f"t_bf{cc}")
            nc.gpsimd.tensor_copy(tb[:], s.t_f[cc][:])
            s.t_bf.append(tb)

        # stats: 4 rows-of-2048 tensors (s0,s1,t0,t1)
        s.mv = pool_st.tile([P, 4, 2], DT.float32, name="mv")
        srcs = [s.s_f[0], s.s_f[1], s.t_f[0], s.t_f[1]]
        for i, x in enumerate(srcs):
            bnst = pool_st.tile([P, 4, 6], DT.float32, name=f"bnst{i}")
            for k in range(4):
                nc.vector.bn_stats(bnst[:, k, :], x[:, 512 * k : 512 * (k + 1)])
            nc.vector.bn_aggr(s.mv[:, i, :], bnst[:])
        # inv = (var * T/(T-1))^-0.5  (eps dropped: |rel err| ~1e-5)
        lnv = pool_st.tile([P, 4], DT.float32, name="lnv")
        nc.scalar.activation(lnv[:], s.mv[:, :, 1], ACTF.Ln, scale=float(T) / (T - 1), bias=cb_zero[:])
        s.inv = pool_st.tile([P, 4], DT.float32, name="inv")
        nc.scalar.activation(s.inv[:], lnv[:], ACTF.Exp, scale=-0.5, bias=cb_zero[:])
        # mi = mean * inv (bf16, matvec rhs for rank-1 bias)
        s.mi = pool_st.tile([P, 4], DT.bfloat16, name="mi")
        nc.vector.tensor_tensor(s.mi[:], s.mv[:, :, 0], s.inv[:], ALU.mult)

        # scaled weights
        s.wq_s = pool_ws.tile([P, NCH * C], DT.bfloat16, name="wq_s")
        s.wk_s = pool_ws.tile([P, NCH * C], DT.bfloat16, name="wk_s")
        for cc in range(NCH):
            nc.vector.tensor_scalar_mul(
                s.wq_s[:, cc * C : (cc + 1) * C],
                wq_bf[:, cc * C : (cc + 1) * C],
                s.inv[:, cc : cc + 1],
            )
            nc.vector.tensor_scalar_mul(
                s.wk_s[:, cc * C : (cc + 1) * C],
                wk_bf[:, cc * C : (cc + 1) * C],
                s.inv[:, 2 + cc : 3 + cc],
            )

    def emit_negb(b):
        """negb[d] = -sum_c w_s[c,d]*(mu[c]*inv[c]) for q (cols 0:2) and
        k (cols 2:4)."""
        s = st[b]
        bps = ps_pr.tile([P, 512], DT.float32, name="prps", tag="prps")
        for wi, w_s in enumerate((s.wq_s, s.wk_s)):
            for dc in range(NCH):
                for cc in range(NCH):
                    nc.tensor.matmul(
                        bps[:, 2 * wi + dc : 2 * wi + dc + 1],
                        lhsT=w_s[:, cc * C + dc * P : cc * C + (dc + 1) * P],
                        rhs=s.mi[:, 2 * wi + cc : 2 * wi + cc + 1],
                        start=(cc == 0),
                        stop=(cc == NCH - 1),
                        skip_group_check=True,
                    )
        s.negb = pool_st.tile([P, 4], DT.float32, name="negb")
        nc.vector.tensor_scalar_mul(s.negb[:], bps[:, 0:4], -1.0)

    def emit_proj_qk(b, which):
        """kt/qt: [d128, dc, t] fp8, bias folded at eviction.
        kt eviction on ACT (identity+bias, in-table), qt on DVE."""
        s = st[b]
        if which == "k":
            w_s, x_bf, boff = s.wk_s, s.t_bf, 2
            dst = s.kt = pool_qk.tile([P, NCH, T], DT.float8e4, name="kt")
        else:
            w_s, x_bf, boff = s.wq_s, s.s_bf, 0
            dst = s.qt = pool_qk.tile([P, NCH, T], DT.float8e4, name="qt")
        for dc in range(NCH):
            for tb in range(4):
                pps = ps_pr.tile([P, 512], DT.float32, name="prps", tag="prps")
                for cc in range(NCH):
                    nc.tensor.matmul(
                        pps[:],
                        lhsT=w_s[:, cc * C + dc * P : cc * C + (dc + 1) * P],
                        rhs=x_bf[cc][:, 512 * tb : 512 * (tb + 1)],
                        start=(cc == 0),
                        stop=(cc == NCH - 1),
                    )
                dsl = dst[:, dc, 512 * tb : 512 * (tb + 1)]
                if which == "k":
                    nc.scalar.activation(
                        dsl, pps[:], ACTF.Identity, bias=s.negb[:, boff + dc : boff + dc + 1]
                    )
                else:
                    nc.vector.tensor_scalar_add(
                        dsl, pps[:], s.negb[:, boff + dc : boff + dc + 1]
                    )

    def emit_proj_v(b):
        """v: [t128, tch, d] fp8 (raw trg @ Wv), v2 = v*v on Pool."""
        s = st[b]
        s.v = pool_v.tile([P, NTCH, C], DT.float8e4, name="v")
        s.v2 = pool_v.tile([P, NTCH, C], DT.float8e4, name="v2")
        for g in range(NPAIR):
            vps = ps_pr.tile([P, 512], DT.float32, name="prps", tag="prps")
            for t2 in range(2):
                tch = 2 * g + t2
                for cc in range(NCH):
                    nc.tensor.matmul(
                        vps[:, 256 * t2 : 256 * (t2 + 1)],
                        lhsT=s.t_bf[cc][:, P * tch : P * (tch + 1)],
                        rhs=wv_bf[:, cc * C : (cc + 1) * C],
                        start=(cc == 0),
                        stop=(cc == NCH - 1),
                        skip_group_check=True,
                    )
            nc.vector.tensor_copy(s.v[:, 2 * g : 2 * g + 2, :], vps[:])
            nc.gpsimd.tensor_mul(
                s.v2[:, 2 * g : 2 * g + 2, :],
                s.v[:, 2 * g : 2 * g + 2, :],
                s.v[:, 2 * g : 2 * g + 2, :],
            )

    def emit_accum_init(b):
        s = st[b]
        s.smc = pool_st.tile([P, NQ, NCH], DT.float32, name="smc")
        s.sm2c = pool_st.tile([P, NQ, NCH], DT.float32, name="sm2c")
        s.av2c = pool_st.tile([P, NQ, NCH], DT.float32, name="av2c")

    def emit_quarter(b, q):
        """scores^T -> exp(fp8) -> {Z, AVu, AV2u} DoubleRow -> fold 1/Z."""
        s = st[b]
        so = SQ * q
        p_all = pool_p.tile([P, NTCH, SQ], DT.float8e4, name="p_all")
        z_ps = ps_z.tile([P, SQ], DT.float32, name="zps", tag="zps")
        avu = [ps_avu.tile([P, SQ], DT.float32, name=f"avu{dc}", tag="avu") for dc in range(NCH)]
        av2u = [ps_av2.tile([P, SQ], DT.float32, name=f"av2u{dc}", tag="av2u") for dc in range(NCH)]

        for jp in range(NPAIR):
            sc = ps_sc.tile([P, 2 * SQ], DT.float32, name="scps", tag="scps")
            for t2 in range(2):
                tch = 2 * jp + t2
                nc.tensor.matmul(
                    sc[:, SQ * t2 : SQ * (t2 + 1)],
                    lhsT=s.kt[:, :, P * tch : P * (tch + 1)],
                    rhs=s.qt[:, :, so : so + SQ],
                    start=True,
                    stop=True,
                    perf_mode=DR,
                )
            # exp(score/16 - 3): the shift keeps fp8e4 < 240 (scores reach ~7.5
            # sigma empirically), cancels via 1/Z
            nc.scalar.activation(
                p_all[:, 2 * jp : 2 * jp + 2, :], sc[:], ACTF.Exp, scale=1.0 / 16.0,
                bias=cb_exp[:],
            )
            pj = p_all[:, 2 * jp : 2 * jp + 2, :]
            nc.tensor.matmul(
                z_ps[:], lhsT=ones_f8[:], rhs=pj,
                start=(jp == 0), stop=(jp == NPAIR - 1),
                perf_mode=DR, skip_group_check=True,
            )
            for dc in range(NCH):
                nc.tensor.matmul(
                    avu[dc][:],
                    lhsT=s.v[:, 2 * jp : 2 * jp + 2, dc * P : (dc + 1) * P],
                    rhs=pj,
                    start=(jp == 0), stop=(jp == NPAIR - 1),
                    perf_mode=DR, skip_group_check=True,
                )
                nc.tensor.matmul(
                    av2u[dc][:],
                    lhsT=s.v2[:, 2 * jp : 2 * jp + 2, dc * P : (dc + 1) * P],
                    rhs=pj,
                    start=(jp == 0), stop=(jp == NPAIR - 1),
                    perf_mode=DR, skip_group_check=True,
                )

        zinv = pool_zi.tile([P, SQ], DT.float32, name="zinv")
        nc.vector.reciprocal(zinv[:], z_ps[:])
        for dc in range(NCH):
            m_t = pool_jk.tile([P, SQ], DT.bfloat16, name="m_t")
            nc.vector.scalar_tensor_tensor(
                out=m_t[:], in0=avu[dc][:], scalar=1.0, in1=zinv[:],
                op0=ALU.mult, op1=ALU.mult,
                accum_out=s.smc[:, q, dc : dc + 1],
            )
            jk2 = pool_jk.tile([P, SQ], DT.bfloat16, name="jk2")
            nc.vector.scalar_tensor_tensor(
                out=jk2[:], in0=m_t[:], scalar=1.0, in1=m_t[:],
                op0=ALU.mult, op1=ALU.mult,
                accum_out=s.sm2c[:, q, dc : dc + 1],
            )
            jk3 = pool_jk.tile([P, SQ], DT.bfloat16, name="jk3")
            nc.vector.scalar_tensor_tensor(
                out=jk3[:], in0=av2u[dc][:], scalar=1.0, in1=zinv[:],
                op0=ALU.mult, op1=ALU.mult,
                accum_out=s.av2c[:, q, dc : dc + 1],
            )

    def emit_finals(b):
        s = st[b]
        # total sums over the 4 quarters -> [P, 2] (col = dc)
        def qsum(cols, nm):
            a = pool_st.tile([P, NCH], DT.float32, name=f"{nm}_a")
            bb = pool_st.tile([P, NCH], DT.float32, name=f"{nm}_b")
            tot = pool_st.tile([P, NCH], DT.float32, name=f"{nm}_t")
            nc.vector.tensor_add(a[:], cols[:, 0, :], cols[:, 1, :])
            nc.vector.tensor_add(bb[:], cols[:, 2, :], cols[:, 3, :])
            nc.vector.tensor_add(tot[:], a[:], bb[:])
            return tot

        sm = qsum(s.smc, "sm")
        sm2 = qsum(s.sm2c, "sm2")
        av2 = qsum(s.av2c, "av2")

        d1 = pool_st.tile([P, NCH], DT.float32, name="d1")
        nc.vector.tensor_sub(d1[:], av2[:], sm2[:])
        r1 = pool_st.tile([P, NCH], DT.float32, name="r1")
        nc.vector.tensor_scalar_max(r1[:], d1[:], 0.0)
        # stdv = sqrt(r1/T) = exp(0.5*ln(r1/T + tiny))
        lns = pool_st.tile([P, NCH], DT.float32, name="lns")
        nc.scalar.activation(lns[:], r1[:], ACTF.Ln, scale=1.0 / T, bias=cb_tiny[:])
        stdv = pool_st.tile([P, NCH], DT.float32, name="stdv")
        nc.scalar.activation(stdv[:], lns[:], ACTF.Exp, scale=0.5, bias=cb_zero[:])

        s.A = pool_st.tile([P, NCH], DT.float32, name="A")
        nc.vector.tensor_tensor(s.A[:], stdv[:], s.inv[:, 0:2], ALU.mult)
        ams = pool_st.tile([P, NCH], DT.float32, name="ams")
        nc.vector.tensor_tensor(ams[:], s.A[:], s.mv[:, 0:2, 0], ALU.mult)
        s.B = pool_st.tile([P, NCH], DT.float32, name="B")
        nc.vector.scalar_tensor_tensor(
            out=s.B[:], in0=sm[:], scalar=1.0 / T, in1=ams[:],
            op0=ALU.mult, op1=ALU.subtract,
        )

    def emit_output(b):
        s = st[b]
        for dc in range(NCH):
            for h in range(2):
                o_sb = pool_out.tile([P, 1024], DT.float32, name="o_sb")
                nc.gpsimd.tensor_scalar(
                    out=o_sb[:],
                    in0=s.s_f[dc][:, 1024 * h : 1024 * (h + 1)],
                    scalar1=s.A[:, dc : dc + 1],
                    scalar2=s.B[:, dc : dc + 1],
                    op0=ALU.mult,
                    op1=ALU.add,
                )
                nc.sync.dma_start(
                    out[b, dc * P : (dc + 1) * P, 1024 * h : 1024 * (h + 1)], o_sb[:]
                )

    # ---------------- emission schedule ----------------
    emit_loads(0)
    emit_loads(1)
    emit_stats_casts(0)
    emit_negb(0)
    emit_proj_qk(0, "k")
    emit_proj_qk(0, "q")
    emit_proj_v(0)
    emit_accum_init(0)
    emit_stats_casts(1)  # DVE/Pool/ACT work overlapping b0 attention
    emit_quarter(0, 0)
    emit_negb(1)
    emit_proj_qk(1, "k")
    emit_quarter(0, 1)
    emit_proj_qk(1, "q")
    emit_quarter(0, 2)
    emit_proj_v(1)
    emit_accum_init(1)
    emit_quarter(0, 3)
    emit_finals(0)
    emit_output(0)
    for q in range(NQ):
        emit_quarter(1, q)
    emit_finals(1)
    emit_output(1)


_NC_CACHE = None


def _get_nc():
    global _NC_CACHE
    if _NC_CACHE is None:
        _NC_CACHE = _build_nc()
    return _NC_CACHE


def _run(src, trg, Wq, Wk, Wv, **kwargs):
    src = np.ascontiguousarray(np.asarray(src, dtype=np.float32))
    trg = np.ascontiguousarray(np.asarray(trg, dtype=np.float32))
    wqt = np.ascontiguousarray(np.asarray(Wq, dtype=np.float32).T)
    wkt = np.ascontiguousarray(np.asarray(Wk, dtype=np.float32).T)
    wvt = np.ascontiguousarray(np.asarray(Wv, dtype=np.float32).T)
    nc = _get_nc()
    in_maps = [
        {
            "src": src[i * B_SH : (i + 1) * B_SH],
            "trg": trg[i * B_SH : (i + 1) * B_SH],
            "wqt": wqt,
            "wkt": wkt,
            "wvt": wvt,
        }
        for i in range(N_CORES)
    ]
    res = run_bass_kernel_spmd(nc, in_maps, list(range(N_CORES)), **kwargs)
    outp = np.concatenate([res.results[i]["out"] for i in range(N_CORES)], axis=0)
    return outp.astype(np.float32), res


def kernel(src, trg, Wq, Wk, Wv):
    outp, _ = _run(src, trg, Wq, Wk, Wv)
    return outp


# revision 5
# speedup vs baseline: 1.3936x; 1.3936x over previous
"""Trainium2 Bass kernel for nn_ChannelAdaptiveNormalization (v2).

Reference computation (per batch):
    src_n = instnorm(src); q = Wq@src_n; k = Wk@instnorm(trg); v = Wv@trg
    attn = softmax(q^T k / sqrt(C))  over t
    mean = attn @ v ; var = relu(attn @ v^2 - mean^2)
    out = sqrt(mean_s[var]) * src_n + mean_s[mean]      (broadcast over time)

v2 design (per-core, data-parallel over batch, 2 batches/core):
  * instance-norm folded into CxC projection weights (column scale by 1/sd,
    rank-1 bias) -- normalized activations never materialize.
  * q/k/p/v/v^2 stored fp8e4; all attention matmuls (scores^T, Z, attn@v,
    attn@v^2) run in DoubleRow perf mode (contract 256 per instruction at
    0.5 cyc/row) -- 2x PE throughput.
  * attention is kept UNNORMALIZED through the PE stages: p = exp(s/16-3)
    (the shift keeps fp8e4 under its +-240 max and cancels in normalization);
    Z via a DoubleRow ones-matmul; 1/Z folded in only at the tiny [d,s]
    AV/AV2 outputs (8x fewer elements than normalizing p).
  * only column-sums over s of M=attn@v, M^2, attn@v^2 are needed; they are
    reduced on the fly from PSUM with accumulate ops.
  * all sqrt computed as exp(0.5*ln(x)) so the ACT engine keeps ONE table
    (natural_log_exp) resident: zero table swaps.
  * final output is a per-(b,c) affine of raw fp32 src: out = A*src + B.
"""

import sys

import numpy as np

if "/opt/trn_rl_repo" not in sys.path:
    sys.path.insert(0, "/opt/trn_rl_repo")

from contextlib import ExitStack

import concourse.bass as bass
import concourse.tile as tile
from concourse import mybir
from concourse.bass_utils import run_bass_kernel_spmd

DT = mybir.dt
ALU = mybir.AluOpType
ACTF = mybir.ActivationFunctionType
DR = mybir.MatmulPerfMode.DoubleRow

N_CORES = 8
B_FULL = 16
B_SH = B_FULL // N_CORES  # 2 batches per core
C = 256
T = 2048
P = 128
NCH = C // P  # 2 channel chunks
NTCH = T // P  # 16 time chunks
NQ = 4  # s-quarters
SQ = T // NQ  # 512 s per quarter
NPAIR = NTCH // 2  # 8 tchunk pairs (DoubleRow contracts 2 chunks)


def _build_nc() -> bass.Bass:
    nc = bass.Bass()
    src = nc.declare_dram_parameter("src", [B_SH, C, T], DT.float32, isOutput=False)
    trg = nc.declare_dram_parameter("trg", [B_SH, C, T], DT.float32, isOutput=False)
    wqt = nc.declare_dram_parameter("wqt", [C, C], DT.float32, isOutput=False)
    wkt = nc.declare_dram_parameter("wkt", [C, C], DT.float32, isOutput=False)
    wvt = nc.declare_dram_parameter("wvt", [C, C], DT.float32, isOutput=False)
    out = nc.declare_dram_parameter("out", [B_SH, C, T], DT.float32, isOutput=True)

    with tile.TileContext(nc) as tc:
        with ExitStack() as ctx:
            _build_kernel(ctx, tc, src, trg, wqt, wkt, wvt, out)
    _legalize_waits(nc)
    return nc


def _legalize_waits(nc: bass.Bass):
    """walrus on this toolchain encodes at most ONE sync wait per
    instruction (NEURON_ISA_TPB_EVENTS has a single wait slot and no
    splitting pass runs).  Hoist all but the last wait of every
    instruction into standalone single-wait EventSemaphore instructions
    on the same engine queue, which preserves ordering semantics."""
    all_sems = {}
    for fn in nc.m.functions:
        for blk in fn.blocks:
            for inst in blk.instructions:
                si = getattr(inst, "sync_info", None)
                if si is None:
                    continue
                for w in list(si.on_wait) + list(si.on_update):
                    if not w.ant_name.startswith("barrier"):
                        all_sems[w.id] = w.ant_name

    for fn in nc.m.functions:
        for blk in fn.blocks:
            snapshot = list(blk.instructions)
            for idx in range(len(snapshot) - 1, -1, -1):
                inst = snapshot[idx]
                if type(inst).__name__ == "InstISA" and getattr(inst, "isa_opcode", None) == 176:
                    # EVENT_SEMAPHORE_RANGE_CLEAR: encoding mismatches this
                    # walrus build; replace with per-sem zero-writes.
                    pos = list(blk.instructions).index(inst)
                    blk.instructions.pop(pos)
                    for sid, sname in sorted(all_sems.items()):
                        ev = mybir.InstEventSemaphore(
                            name=nc.get_next_instruction_name(), ins=[], outs=[]
                        )
                        ev.engine = inst.engine
                        ev.sync_info = mybir.SyncInfo(
                            on_wait=[],
                            on_update=[
                                mybir.SyncUpdate(
                                    sync_type="semaphore",
                                    id=sid,
                                    ant_name=sname,
                                    update_mode="sem-wr-imm",
                                    update_value=0,
                                )
                            ],
                        )
                        nc.register_instruction(ev)
                        blk.instructions.insert(pos, ev)
                        pos += 1

    for fn in nc.m.functions:
        for blk in fn.blocks:
            snapshot = list(blk.instructions)
            for idx in range(len(snapshot) - 1, -1, -1):
                inst = snapshot[idx]
                si = getattr(inst, "sync_info", None)
                if si is None or len(si.on_wait) <= 1:
                    continue
                waits = list(si.on_wait)
                evs = []
                for w in waits[:-1]:
                    ev = mybir.InstEventSemaphore(
                        name=nc.get_next_instruction_name(), ins=[], outs=[]
                    )
                    ev.engine = inst.engine
                    ev.sync_info = mybir.SyncInfo(on_wait=[w], on_update=[])
                    nc.register_instruction(ev)
                    evs.append(ev)
                si.on_wait = waits[-1:]
                inst.sync_info = si
                for ev in reversed(evs):
                    blk.instructions.insert(idx, ev)


def _build_kernel(ctx, tc, src, trg, wqt, wkt, wvt, out):
    nc = tc.nc
    ep = ctx.enter_context

    # ---------------- pools ----------------
    pool_const = ep(tc.tile_pool(name="const", bufs=1))
    pool_io = ep(tc.tile_pool(name="io", bufs=2))        # fp32 inputs
    pool_bf = ep(tc.tile_pool(name="bfx", bufs=2))       # bf16 activations
    pool_qk = ep(tc.tile_pool(name="qk", bufs=2))        # fp8 q/k
    pool_v = ep(tc.tile_pool(name="vp", bufs=2))         # fp8 v/v2
    pool_p = ep(tc.tile_pool(name="pp", bufs=2))         # fp8 exp scores
    pool_st = ep(tc.tile_pool(name="st", bufs=2))        # small stats
    pool_ws = ep(tc.tile_pool(name="ws", bufs=2))        # scaled weights
    pool_zi = ep(tc.tile_pool(name="zi", bufs=2))        # zinv
    pool_jk = ep(tc.tile_pool(name="jk", bufs=2))        # junk evict targets
    pool_out = ep(tc.tile_pool(name="oo", bufs=2))       # output staging
    pool_wtmp = ep(tc.tile_pool(name="wtmp", bufs=3))

    ps_sc = ep(tc.tile_pool(name="ps_sc", bufs=1, space="PSUM"))   # scores [P,1024]
    ps_z = ep(tc.tile_pool(name="ps_z", bufs=1, space="PSUM"))     # Z [P,512]
    ps_avu = ep(tc.tile_pool(name="ps_avu", bufs=2, space="PSUM"))  # AVu per dc
    ps_av2 = ep(tc.tile_pool(name="ps_av2", bufs=2, space="PSUM"))  # AV2u per dc
    ps_pr = ep(tc.tile_pool(name="ps_pr", bufs=1, space="PSUM"))   # projections

    # ---------------- constants / weights (once) ----------------
    ones_f8 = pool_const.tile([P, 2, P], DT.float8e4, name="ones_f8")
    nc.vector.memset(ones_f8[:], 1.0)
    cb_exp = pool_const.tile([P, 1], DT.float32, name="cb_exp")
    nc.vector.memset(cb_exp[:], -3.0)
    cb_tiny = pool_const.tile([P, 1], DT.float32, name="cb_tiny")
    nc.vector.memset(cb_tiny[:], 1e-30)
    cb_zero = pool_const.tile([P, 1], DT.float32, name="cb_zero")
    nc.vector.memset(cb_zero[:], 0.0)

    # weight layout in SBUF: [128 (c within chunk), cc*256 + d]
    wq_bf = pool_const.tile([P, NCH * C], DT.bfloat16, name="wq_bf")
    wk_bf = pool_const.tile([P, NCH * C], DT.bfloat16, name="wk_bf")
    wv_bf = pool_const.tile([P, NCH * C], DT.bfloat16, name="wv_bf")
    for w_bf, w_d in ((wq_bf, wqt), (wk_bf, wkt), (wv_bf, wvt)):
        wtmp = pool_wtmp.tile([P, NCH * C], DT.float32, name="wtmp")
        nc.gpsimd.dma_start(
            wtmp[:].rearrange("p (a d) -> p a d", a=NCH),
            w_d[:].rearrange("(a p) d -> p a d", p=P),
        )
        nc.vector.tensor_copy(w_bf[:], wtmp[:])

    # ---------------- per-batch state ----------------
    class BState:
        pass

    st = [BState() for _ in range(B_SH)]

    def emit_loads(b):
        """trg first (gates the kt path), [128,1024] chunks, issued on SP."""
        s = st[b]
        s.s_f, s.t_f = [], []
        for cc in range(NCH):
            tf = pool_io.tile([P, T], DT.float32, name=f"t_f{cc}")
            s.t_f.append(tf)
        for cc in range(NCH):
            sf = pool_io.tile([P, T], DT.float32, name=f"s_f{cc}")
            s.s_f.append(sf)
        for cc in range(NCH):
            for h in range(2):
                sl = slice(1024 * h, 1024 * (h + 1))
                nc.sync.dma_start(s.t_f[cc][:, sl], trg[b, cc * P : (cc + 1) * P, sl])
        for cc in range(NCH):
            for h in range(2):
                sl = slice(1024 * h, 1024 * (h + 1))
                nc.sync.dma_start(s.s_f[cc][:, sl], src[b, cc * P : (cc + 1) * P, sl])

    def emit_stats_casts(b):
        """bn stats on fp32 inputs, rsqrt via ln/exp on ACT; t casts on DVE
        (kt path is critical), s casts on Pool (off critical path)."""
        s = st[b]
        # trg stats first
        s.mv = pool_st.tile([P, 4, 2], DT.float32, name="mv")
        for i, x in ((2, s.t_f[0]), (3, s.t_f[1])):
            bnst = pool_st.tile([P, 4, 6], DT.float32, name=f"bnst{i}")
            for k in range(4):
                nc.vector.bn_stats(bnst[:, k, :], x[:, 512 * k : 512 * (k + 1)])
            nc.vector.bn_aggr(s.mv[:, i, :], bnst[:])
        # t casts on DVE, halves for pipelining
        s.t_bf = []
        for cc in range(NCH):
            tb = pool_bf.tile([P, T], DT.bfloat16, name=f"t_bf{cc}")
            for h in range(2):
                sl = slice(1024 * h, 1024 * (h + 1))
                nc.vector.tensor_copy(tb[:, sl], s.t_f[cc][:, sl])
            s.t_bf.append(tb)
        for i, x in ((0, s.s_f[0]), (1, s.s_f[1])):
            bnst = pool_st.tile([P, 4, 6], DT.float32, name=f"bnst{i}")
            for k in range(4):
                nc.vector.bn_stats(bnst[:, k, :], x[:, 512 * k : 512 * (k + 1)])
            nc.vector.bn_aggr(s.mv[:, i, :], bnst[:])
        # s casts on Pool
        s.s_bf = []
        for cc in range(NCH):
            sb = pool_bf.tile([P, T], DT.bfloat16, name=f"s_bf{cc}")
            for h in range(2):
                sl = slice(1024 * h, 1024 * (h + 1))
                nc.gpsimd.tensor_copy(sb[:, sl], s.s_f[cc][:, sl])
            s.s_bf.append(sb)
        # inv = (var * T/(T-1))^-0.5  (eps dropped: |rel err| ~1e-5)
        lnv = pool_st.tile([P, 4], DT.float32, name="lnv")
        nc.scalar.activation(lnv[:], s.mv[:, :, 1], ACTF.Ln, scale=float(T) / (T - 1), bias=cb_zero[:])
        s.inv = pool_st.tile([P, 4], DT.float32, name="inv")
        nc.scalar.activation(s.inv[:], lnv[:], ACTF.Exp, scale=-0.5, bias=cb_zero[:])
        # mi = mean * inv (bf16, matvec rhs for rank-1 bias)
        s.mi = pool_st.tile([P, 4], DT.bfloat16, name="mi")
        nc.vector.tensor_tensor(s.mi[:], s.mv[:, :, 0], s.inv[:], ALU.mult)

        # scaled weights (k first: kt path gates attention)
        s.wq_s = pool_ws.tile([P, NCH * C], DT.bfloat16, name="wq_s")
        s.wk_s = pool_ws.tile([P, NCH * C], DT.bfloat16, name="wk_s")
        for cc in range(NCH):
            nc.vector.tensor_scalar_mul(
                s.wk_s[:, cc * C : (cc + 1) * C],
                wk_bf[:, cc * C : (cc + 1) * C],
                s.inv[:, 2 + cc : 3 + cc],
            )
        for cc in range(NCH):
            nc.vector.tensor_scalar_mul(
                s.wq_s[:, cc * C : (cc + 1) * C],
                wq_bf[:, cc * C : (cc + 1) * C],
                s.inv[:, cc : cc + 1],
            )

    def emit_negb(b):
        """negb[d] = -sum_c w_s[c,d]*(mu[c]*inv[c]) for q (cols 0:2) and
        k (cols 2:4)."""
        s = st[b]
        bps = ps_pr.tile([P, 512], DT.float32, name="prps", tag="prps")
        for wi, w_s in enumerate((s.wq_s, s.wk_s)):
            for dc in range(NCH):
                for cc in range(NCH):
                    nc.tensor.matmul(
                        bps[:, 2 * wi + dc : 2 * wi + dc + 1],
                        lhsT=w_s[:, cc * C + dc * P : cc * C + (dc + 1) * P],
                        rhs=s.mi[:, 2 * wi + cc : 2 * wi + cc + 1],
                        start=(cc == 0),
                        stop=(cc == NCH - 1),
                        skip_group_check=True,
                    )
        s.negb = pool_st.tile([P, 4], DT.float32, name="negb")
        nc.vector.tensor_scalar_mul(s.negb[:], bps[:, 0:4], -1.0)

    def emit_proj_qk(b, which):
        """kt/qt: [d128, dc, t] fp8, bias folded at eviction.
        kt eviction on ACT (identity+bias, in-table), qt on DVE."""
        s = st[b]
        if which == "k":
            w_s, x_bf, boff = s.wk_s, s.t_bf, 2
            dst = s.kt = pool_qk.tile([P, NCH, T], DT.float8e4, name="kt")
        else:
            w_s, x_bf, boff = s.wq_s, s.s_bf, 0
            dst = s.qt = pool_qk.tile([P, NCH, T], DT.float8e4, name="qt")
        for dc in range(NCH):
            for tb in range(4):
                pps = ps_pr.tile([P, 512], DT.float32, name="prps", tag="prps")
                for cc in range(NCH):
                    nc.tensor.matmul(
                        pps[:],
                        lhsT=w_s[:, cc * C + dc * P : cc * C + (dc + 1) * P],
                        rhs=x_bf[cc][:, 512 * tb : 512 * (tb + 1)],
                        start=(cc == 0),
                        stop=(cc == NCH - 1),
                    )
                dsl = dst[:, dc, 512 * tb : 512 * (tb + 1)]
                if which == "k":
                    nc.scalar.activation(
                        dsl, pps[:], ACTF.Identity, bias=s.negb[:, boff + dc : boff + dc + 1]
                    )
                else:
                    nc.vector.tensor_scalar_add(
                        dsl, pps[:], s.negb[:, boff + dc : boff + dc + 1]
                    )

    def emit_proj_v(b):
        """v: [t128, tch, d] fp8 (raw trg @ Wv), v2 = v*v on Pool."""
        s = st[b]
        s.v = pool_v.tile([P, NTCH, C], DT.float8e4, name="v")
        s.v2 = pool_v.tile([P, NTCH, C], DT.float8e4, name="v2")
        for g in range(NPAIR):
            vps = ps_pr.tile([P, 512], DT.float32, name="prps", tag="prps")
            for t2 in range(2):
                tch = 2 * g + t2
                for cc in range(NCH):
                    nc.tensor.matmul(
                        vps[:, 256 * t2 : 256 * (t2 + 1)],
                        lhsT=s.t_bf[cc][:, P * tch : P * (tch + 1)],
                        rhs=wv_bf[:, cc * C : (cc + 1) * C],
                        start=(cc == 0),
                        stop=(cc == NCH - 1),
                        skip_group_check=True,
                    )
            nc.vector.tensor_copy(s.v[:, 2 * g : 2 * g + 2, :], vps[:])
            nc.gpsimd.tensor_mul(
                s.v2[:, 2 * g : 2 * g + 2, :],
                s.v[:, 2 * g : 2 * g + 2, :],
                s.v[:, 2 * g : 2 * g + 2, :],
            )

    def emit_accum_init(b):
        s = st[b]
        s.smc = pool_st.tile([P, NQ, NCH], DT.float32, name="smc")
        s.sm2c = pool_st.tile([P, NQ, NCH], DT.float32, name="sm2c")
        s.av2c = pool_st.tile([P, NQ, NCH], DT.float32, name="av2c")

    def emit_quarter(b, q):
        """scores^T -> exp(fp8) -> {Z, AVu, AV2u} DoubleRow -> fold 1/Z."""
        s = st[b]
        so = SQ * q
        p_all = pool_p.tile([P, NTCH, SQ], DT.float8e4, name="p_all")
        z_ps = ps_z.tile([P, SQ], DT.float32, name="zps", tag="zps")
        avu = [ps_avu.tile([P, SQ], DT.float32, name=f"avu{dc}", tag="avu") for dc in range(NCH)]
        av2u = [ps_av2.tile([P, SQ], DT.float32, name=f"av2u{dc}", tag="av2u") for dc in range(NCH)]

        for jp in range(NPAIR):
            sc = ps_sc.tile([P, 2 * SQ], DT.float32, name="scps", tag="scps")
            for t2 in range(2):
                tch = 2 * jp + t2
                nc.tensor.matmul(
                    sc[:, SQ * t2 : SQ * (t2 + 1)],
                    lhsT=s.kt[:, :, P * tch : P * (tch + 1)],
                    rhs=s.qt[:, :, so : so + SQ],
                    start=True,
                    stop=True,
                    perf_mode=DR,
                )
            # exp(score/16 - 3): the shift keeps fp8e4 < 240 (scores reach ~7.5
            # sigma empirically), cancels via 1/Z
            nc.scalar.activation(
                p_all[:, 2 * jp : 2 * jp + 2, :], sc[:], ACTF.Exp, scale=1.0 / 16.0,
                bias=cb_exp[:],
            )
            pj = p_all[:, 2 * jp : 2 * jp + 2, :]
            nc.tensor.matmul(
                z_ps[:], lhsT=ones_f8[:], rhs=pj,
                start=(jp == 0), stop=(jp == NPAIR - 1),
                perf_mode=DR, skip_group_check=True,
            )
            for dc in range(NCH):
                nc.tensor.matmul(
                    avu[dc][:],
                    lhsT=s.v[:, 2 * jp : 2 * jp + 2, dc * P : (dc + 1) * P],
                    rhs=pj,
                    start=(jp == 0), stop=(jp == NPAIR - 1),
                    perf_mode=DR, skip_group_check=True,
                )
                nc.tensor.matmul(
                    av2u[dc][:],
                    lhsT=s.v2[:, 2 * jp : 2 * jp + 2, dc * P : (dc + 1) * P],
                    rhs=pj,
                    start=(jp == 0), stop=(jp == NPAIR - 1),
                    perf_mode=DR, skip_group_check=True,
                )

        zinv = pool_zi.tile([P, SQ], DT.float32, name="zinv")
        nc.vector.reciprocal(zinv[:], z_ps[:])
        for dc in range(NCH):
            m_t = pool_jk.tile([P, SQ], DT.bfloat16, name="m_t")
            nc.vector.scalar_tensor_tensor(
                out=m_t[:], in0=avu[dc][:], scalar=1.0, in1=zinv[:],
                op0=ALU.mult, op1=ALU.mult,
                accum_out=s.smc[:, q, dc : dc + 1],
            )
            jk2 = pool_jk.tile([P, SQ], DT.bfloat16, name="jk2")
            nc.vector.scalar_tensor_tensor(
                out=jk2[:], in0=m_t[:], scalar=1.0, in1=m_t[:],
                op0=ALU.mult, op1=ALU.mult,
                accum_out=s.sm2c[:, q, dc : dc + 1],
            )
            jk3 = pool_jk.tile([P, SQ], DT.bfloat16, name="jk3")
            nc.vector.scalar_tensor_tensor(
                out=jk3[:], in0=av2u[dc][:], scalar=1.0, in1=zinv[:],
                op0=ALU.mult, op1=ALU.mult,
                accum_out=s.av2c[:, q, dc : dc + 1],
            )

    def emit_finals(b):
        s = st[b]
        # total sums over the 4 quarters -> [P, 2] (col = dc)
        def qsum(cols, nm):
            a = pool_st.tile([P, NCH], DT.float32, name=f"{nm}_a")
            bb = pool_st.tile([P, NCH], DT.float32, name=f"{nm}_b")
            tot = pool_st.tile([P, NCH], DT.float32, name=f"{nm}_t")
            nc.vector.tensor_add(a[:], cols[:, 0, :], cols[:, 1, :])
            nc.vector.tensor_add(bb[:], cols[:, 2, :], cols[:, 3, :])
            nc.vector.tensor_add(tot[:], a[:], bb[:])
            return tot

        sm = qsum(s.smc, "sm")
        sm2 = qsum(s.sm2c, "sm2")
        av2 = qsum(s.av2c, "av2")

        d1 = pool_st.tile([P, NCH], DT.float32, name="d1")
        nc.vector.tensor_sub(d1[:], av2[:], sm2[:])
        r1 = pool_st.tile([P, NCH], DT.float32, name="r1")
        nc.vector.tensor_scalar_max(r1[:], d1[:], 0.0)
        # stdv = sqrt(r1/T) = exp(0.5*ln(r1/T + tiny))
        lns = pool_st.tile([P, NCH], DT.float32, name="lns")
        nc.scalar.activation(lns[:], r1[:], ACTF.Ln, scale=1.0 / T, bias=cb_tiny[:])
        stdv = pool_st.tile([P, NCH], DT.float32, name="stdv")
        nc.scalar.activation(stdv[:], lns[:], ACTF.Exp, scale=0.5, bias=cb_zero[:])

        s.A = pool_st.tile([P, NCH], DT.float32, name="A")
        nc.vector.tensor_tensor(s.A[:], stdv[:], s.inv[:, 0:2], ALU.mult)
        ams = pool_st.tile([P, NCH], DT.float32, name="ams")
        nc.vector.tensor_tensor(ams[:], s.A[:], s.mv[:, 0:2, 0], ALU.mult)
        s.B = pool_st.tile([P, NCH], DT.float32, name="B")
        nc.vector.scalar_tensor_tensor(
            out=s.B[:], in0=sm[:], scalar=1.0 / T, in1=ams[:],
            op0=ALU.mult, op1=ALU.subtract,
        )

    def emit_output(b):
        s = st[b]
        for dc in range(NCH):
            for h in range(2):
                o_sb = pool_out.tile([P, 1024], DT.float32, name="o_sb")
                nc.gpsimd.tensor_scalar(
                    out=o_sb[:],
                    in0=s.s_f[dc][:, 1024 * h : 1024 * (h + 1)],
                    scalar1=s.A[:, dc : dc + 1],
                    scalar2=s.B[:, dc : dc + 1],
                    op0=ALU.mult,
                    op1=ALU.add,
                )
                nc.sync.dma_start(
                    out[b, dc * P : (dc + 1) * P, 1024 * h : 1024 * (h + 1)], o_sb[:]
                )

    # ---------------- emission schedule ----------------
    emit_loads(0)
    emit_stats_casts(0)
    emit_negb(0)
    emit_proj_qk(0, "k")
    emit_proj_qk(0, "q")
    emit_proj_v(0)
    emit_accum_init(0)
    emit_loads(1)
    emit_quarter(0, 0)
    emit_stats_casts(1)
    emit_quarter(0, 1)
    emit_negb(1)
    emit_proj_qk(1, "k")
    emit_quarter(0, 2)
    emit_proj_qk(1, "q")
    emit_proj_v(1)
    emit_accum_init(1)
    emit_quarter(0, 3)
    emit_finals(0)
    emit_output(0)
    for q in range(NQ):
        emit_quarter(1, q)
    emit_finals(1)
    emit_output(1)


_NC_CACHE = None


def _get_nc():
    global _NC_CACHE
    if _NC_CACHE is None:
        _NC_CACHE = _build_nc()
    return _NC_CACHE


def _run(src, trg, Wq, Wk, Wv, **kwargs):
    src = np.ascontiguousarray(np.asarray(src, dtype=np.float32))
    trg = np.ascontiguousarray(np.asarray(trg, dtype=np.float32))
    wqt = np.ascontiguousarray(np.asarray(Wq, dtype=np.float32).T)
    wkt = np.ascontiguousarray(np.asarray(Wk, dtype=np.float32).T)
    wvt = np.ascontiguousarray(np.asarray(Wv, dtype=np.float32).T)
    nc = _get_nc()
    in_maps = [
        {
            "src": src[i * B_SH : (i + 1) * B_SH],
            "trg": trg[i * B_SH : (i + 1) * B_SH],
            "wqt": wqt,
            "wkt": wkt,
            "wvt": wvt,
        }
        for i in range(N_CORES)
    ]
    res = run_bass_kernel_spmd(nc, in_maps, list(range(N_CORES)), **kwargs)
    outp = np.concatenate([res.results[i]["out"] for i in range(N_CORES)], axis=0)
    return outp.astype(np.float32), res


def kernel(src, trg, Wq, Wk, Wv):
    outp, _ = _run(src, trg, Wq, Wk, Wv)
    return outp
